# revision 1
# baseline (speedup 1.0000x reference)
"""Trainium2 Bass kernel for nn_MultiHeadFast (multi-head attention with
softmax over the QUERY axis).

Math (faithful to the reference):
  qkv = x @ Ws;  per (b,h):  S[q,k] = Q.K^T,  causal mask k<=q,
  P = softmax_over_q(S * T^-0.5),  out = P @ V.

Key layout trick: compute S TRANSPOSED (S^T[k,q], keys on partitions) so the
query-axis softmax is a free-axis reduction, and S^T is exactly the lhsT
operand needed for out^T = V^T @ P.  The normalizer (per key k) is folded
into V's rows before the PV matmul.  exp has no max-subtraction: |S*c| < 1.5.

Sharding: tensor-parallel over heads.  Core c owns heads {2c, 2c+1}; its Ws
column slice is passed from the host; no collectives.  Everything is bf16
with fp32 accumulation (measured ~5e-3 L2 error vs the fp32 reference).
"""

import numpy as np
from contextlib import ExitStack

import concourse.bass as bass
import concourse.mybir as mybir
import concourse.tile as tile
from concourse import bacc
from concourse.bass_utils import run_bass_kernel_spmd
from concourse.masks import make_identity

B, T, E = 2, 2048, 1024
H, D = 16, 64
NCORES = 8
HPC = H // NCORES            # heads per core = 2
FPC = HPC * D                # feature cols per core per Q/K/V = 128
P = 128
NT = B * T                   # 4096 tokens total
EK = E // P                  # 8 contraction blocks for QKV
NSLAB = T // 512             # 4 query slabs per batch
KTILES = T // P              # 16 key tiles per batch
DT = mybir.dt.bfloat16
F32 = mybir.dt.float32
SCALE = float(T) ** -0.5
NEG = -1e30


def build_kernel():
    nc = bacc.Bacc("TRN2", target_bir_lowering=False, debug=False)
    x_dram = nc.dram_tensor("x", (NT, E), F32, kind="ExternalInput")
    w_dram = nc.dram_tensor("wsl", (E, 3 * FPC), F32, kind="ExternalInput")
    out_dram = nc.dram_tensor("out", (B, T, FPC), F32, kind="ExternalOutput")

    with tile.TileContext(nc) as tc, ExitStack() as ctx:
        const = ctx.enter_context(tc.tile_pool(name="const", bufs=1))
        xtp = ctx.enter_context(tc.tile_pool(name="xtp", bufs=1))
        qkvp = ctx.enter_context(tc.tile_pool(name="qkvp", bufs=1))
        work = ctx.enter_context(tc.tile_pool(name="work", bufs=2))
        strips = ctx.enter_context(tc.tile_pool(name="strips", bufs=4))
        small = ctx.enter_context(tc.tile_pool(name="small", bufs=8))
        outp = ctx.enter_context(tc.tile_pool(name="outp", bufs=3))
        ps = ctx.enter_context(tc.tile_pool(name="ps", bufs=2, space="PSUM"))
        dram = ctx.enter_context(tc.tile_pool(name="dram", bufs=1, space="DRAM"))

        # ---- constants ----
        id_bf = const.tile([P, P], DT, name="id_bf")
        make_identity(nc, id_bf)
        id_f32 = const.tile([P, P], F32, name="id_f32")
        make_identity(nc, id_f32)
        zeros_bf = const.tile([P, P], DT, name="zeros_bf")
        nc.gpsimd.memset(zeros_bf[:], 0.0)
        # diagmask[p, f] = 0 if f >= p else NEG   (keys on partitions, q free)
        diagmask = const.tile([P, P], F32, name="diagmask")
        nc.gpsimd.memset(diagmask[:], 0.0)
        nc.gpsimd.affine_select(
            out=diagmask[:],
            in_=diagmask[:],
            compare_op=mybir.AluOpType.is_ge,
            fill=NEG,
            base=0,
            pattern=[[1, P]],
            channel_multiplier=-1,
        )

        # ---- phase A: x^T (bf16) via cast-DMA + DMA transpose ----
        # phase A strategy: load x fp32 natural (the only input DMA, 16MB),
        # cast to bf16 on GpSimd, transpose 128x128 blocks on the PE, and
        # interleave the QKV matmuls per 512-token slab as x^T becomes ready.
        wsl_f32 = qkvp.tile([P, EK, 3 * FPC], F32, name="wsl_f32")
        nc.sync.dma_start(wsl_f32[:], w_dram.rearrange("(eo ei) f -> ei eo f", ei=P))
        wsl = qkvp.tile([P, EK, 3 * FPC], DT, name="wsl")
        nc.vector.tensor_copy(wsl[:], wsl_f32[:])

        xT = xtp.tile([P, EK, NT], DT, name="xT")
        qt = qkvp.tile([P, NT], DT, name="qt")
        kt_sb = qkvp.tile([P, NT], DT, name="kt_sb")
        vt = qkvp.tile([P, NT], DT, name="vt")
        dsts = [qt, kt_sb, vt]
        for n in range(NT // 512):  # 512-token slabs
            xb = work.tile([P, 4, E], DT, tag="xb", bufs=2, name="xb")
            # SWDGE cast-DMA: fp32 DRAM -> bf16 SBUF, tokens on partitions
            nc.gpsimd.dma_start(
                out=xb[:],
                in_=x_dram[512 * n : 512 * (n + 1), :].rearrange(
                    "(w p) e -> p w e", p=P
                ),
            )
            for w in range(4):
                tp = ps.tile([P, E], DT, tag="pv", bufs=4, name="xtp")
                for e in range(EK):
                    nc.tensor.transpose(
                        tp[:, e * P : (e + 1) * P], xb[:, w, e * P : (e + 1) * P], id_bf[:]
                    )
                cp = nc.scalar.copy if w % 2 == 0 else nc.vector.tensor_copy
                cp(
                    xT[:, :, n * 512 + w * P : n * 512 + (w + 1) * P],
                    tp.rearrange("p (e c) -> p e c", c=P),
                )
            for m in range(3):
                mm_ps = ps.tile([P, 512], F32, tag="pv", bufs=4, name="qkv_ps")
                for e in range(EK):
                    nc.tensor.matmul(
                        mm_ps[:],
                        lhsT=wsl[:, e, m * P : (m + 1) * P],
                        rhs=xT[:, e, n * 512 : (n + 1) * 512],
                        start=(e == 0),
                        stop=(e == EK - 1),
                    )
                nc.scalar.copy(dsts[m][:, n * 512 : (n + 1) * 512], mm_ps[:])

        # ---- phase C: V^T -> V (tokens on partitions), per (b, hh) ----
        v_nat = qkvp.tile([P, B * HPC, KTILES, D], DT, name="v_nat")
        for b in range(B):
            for hh in range(HPC):
                for k in range(KTILES):
                    tok0 = b * T + k * P
                    tps = ps.tile([P, D], DT, tag="pv", bufs=4, name="vtp")
                    nc.tensor.transpose(
                        tps[:],
                        vt[hh * D : (hh + 1) * D, tok0 : tok0 + P],
                        id_bf[hh * D : (hh + 1) * D, hh * D : (hh + 1) * D],
                    )
                    nc.vector.tensor_copy(v_nat[:, b * HPC + hh, k, :], tps[:])

        # ---- phase D: attention per batch (software-pipelined over k) ----
        for b in range(B):
            pv_ps = [
                ps.tile([P, 512], F32, tag="pv", bufs=4, name=f"pv_{b}_{j}")
                for j in range(NSLAB)
            ]
            # Zero-initialize each PV accumulator bank with a full-width
            # zero matmul so every partition row's has_written state is set
            # identically under both the per-row and whole-bank semantics;
            # all real PV matmuls then accumulate with start=False.
            for j in range(NSLAB):
                nc.tensor.matmul(
                    pv_ps[j][:],
                    lhsT=zeros_bf[:],
                    rhs=qt[:, b * T : b * T + 512],
                    start=True,
                    stop=False,
                    skip_group_check=True,
                )

            def chunk_mms(b, k, hh, strip, coff, cw):
                """S^T matmuls + mask + exp for one chunk of a head strip."""
                j0 = k // 4
                q0 = 512 * j0
                dead = P * k - q0
                sps = ps.tile([P, 1024], F32, tag="sps", bufs=2, name="sps")
                for so in range(0, cw, 512):
                    qs = q0 + coff + so
                    nc.tensor.matmul(
                        sps[:, so : so + 512],
                        lhsT=kt_sb[hh * D : (hh + 1) * D, b * T + k * P : b * T + k * P + P],
                        rhs=qt[hh * D : (hh + 1) * D, b * T + qs : b * T + qs + 512],
                        start=True,
                        stop=True,
                    )
                acc = small.tile([P, 1], F32, tag="acc", name="acc")
                if coff == 0:
                    nc.vector.tensor_add(
                        sps[:, dead : dead + P], sps[:, dead : dead + P], diagmask[:]
                    )
                    if dead > 0:
                        nc.gpsimd.memset(strip[:, 0:dead], 0.0)
                    nc.scalar.activation(
                        strip[:, dead:cw],
                        sps[:, dead:cw],
                        mybir.ActivationFunctionType.Exp,
                        scale=SCALE,
                        accum_out=acc[:],
                    )
                else:
                    nc.scalar.activation(
                        strip[:, coff : coff + cw],
                        sps[:, :cw],
                        mybir.ActivationFunctionType.Exp,
                        scale=SCALE,
                        accum_out=acc[:],
                    )
                return acc

            def finish_head(b, k, hh, partials):
                if len(partials) == 1:
                    ssum = partials[0]
                else:
                    ssum = small.tile([P, 1], F32, tag="acc", name="ssum")
                    nc.vector.tensor_add(ssum[:], partials[0][:], partials[1][:])
                rsum = small.tile([P, 1], F32, tag="acc", name="rsum")
                nc.vector.reciprocal(rsum[:], ssum[:])
                vp = small.tile([P, D], DT, tag="vp", name="vp")
                nc.vector.tensor_scalar_mul(
                    vp[:], v_nat[:, b * HPC + hh, k, :], rsum[:]
                )
                return vp

            def pv_head(b, k, hh, strip, vp):
                j0 = k // 4
                q0 = 512 * j0
                for j in range(j0, NSLAB):
                    nc.tensor.matmul(
                        pv_ps[j][hh * D : (hh + 1) * D, :],
                        lhsT=vp[:],
                        rhs=strip[:, 512 * j - q0 : 512 * j - q0 + 512],
                        start=False,
                        stop=(k == 4 * j + 3 and hh == HPC - 1),
                        skip_group_check=True,
                    )

            # software pipeline: chunk-level head alternation keeps 2 chunks
            # in flight (one per head) so the ACT exp stream never starves;
            # PV matmuls of k-1 fill the PE between chunk groups.
            prev = {}
            for k in range(KTILES):
                j0 = k // 4
                L = T - 512 * j0
                strip_k = {}
                parts = {0: [], 1: []}
                for hh in range(HPC):
                    strip_k[hh] = strips.tile([P, T], DT, tag="strip", name=f"s{hh}")
                coff = 0
                while coff < L:
                    cw = min(1024, L - coff)
                    for hh in range(HPC):
                        parts[hh].append(chunk_mms(b, k, hh, strip_k[hh], coff, cw))
                    coff += cw
                for hh in range(HPC):
                    vp = finish_head(b, k, hh, parts[hh])
                    if k > 0:
                        pv_head(b, k - 1, hh, *prev[hh])
                    prev[hh] = (strip_k[hh], vp)
            for hh in range(HPC):
                pv_head(b, KTILES - 1, hh, *prev[hh])
            # evacuate + transpose out^T -> out
            for j in range(NSLAB):
                osb = outp.tile([P, 512], F32, tag="osb", name="osb")
                nc.vector.tensor_copy(osb[:], pv_ps[j][:])
                o_sb = outp.tile([P, 4, P], F32, tag="o_sb", name="o_sb")
                for w in range(4):
                    tp = ps.tile([P, P], F32, tag="pv", bufs=4, name="otp")
                    nc.tensor.transpose(tp[:], osb[:, w * P : (w + 1) * P], id_f32[:])
                    nc.vector.tensor_copy(o_sb[:, w, :], tp[:])
                nc.sync.dma_start(
                    out_dram[b, 512 * j : 512 * (j + 1), :].rearrange(
                        "(w p) f -> p w f", p=P
                    ),
                    o_sb[:],
                )
    nc.compile()
    return nc


_NC_CACHE = None


def kernel(x: np.ndarray, Ws: np.ndarray) -> np.ndarray:
    global _NC_CACHE
    if _NC_CACHE is None:
        _NC_CACHE = build_kernel()
    nc = _NC_CACHE

    x2 = np.ascontiguousarray(x.reshape(NT, E).astype(np.float32, copy=False))
    in_maps = []
    for c in range(NCORES):
        cols = np.concatenate(
            [
                Ws[:, c * FPC : (c + 1) * FPC],
                Ws[:, E + c * FPC : E + (c + 1) * FPC],
                Ws[:, 2 * E + c * FPC : 2 * E + (c + 1) * FPC],
            ],
            axis=1,
        ).astype(np.float32, copy=False)
        in_maps.append({"x": x2, "wsl": np.ascontiguousarray(cols)})

    res = run_bass_kernel_spmd(nc, in_maps, core_ids=list(range(NCORES)))
    out = np.empty((B, T, H * D), np.float32)
    for c in range(NCORES):
        out[:, :, c * FPC : (c + 1) * FPC] = res.results[c]["out"]
    return out



# revision 5
# speedup vs baseline: 1.4769x; 1.4769x over previous
"""Trainium2 Bass kernel for nn_MultiHeadFast (multi-head attention with
softmax over the QUERY axis).

Math (faithful to the reference):
  qkv = x @ Ws;  per (b,h):  S[q,k] = Q.K^T,  causal mask k<=q,
  P = softmax_over_q(S * T^-0.5),  out = P @ V.

Layout strategy (v2):
  - Host passes x already TRANSPOSED and cast to bf16 (xT: [E, B*T]) so no
    on-chip transposes of x are needed (the old kernel burned ~70us of PE
    time there).  Host also un-transposes the output (kernel writes out^T).
  - Per core (2 heads): qkv^T = Ws_slice^T x^T via PE matmuls (N=512).
  - S is computed TRANSPOSED (S^T[k,q], keys on partitions) so the
    query-axis softmax is a free-axis reduction fused into the exp
    (accum_out), and S^T is the rhs for out^T = vp^T ... PV matmuls.
  - V natural (tokens on partitions) via 32 full 128x128 PE transposes.
  - exp runs on ScalarE straight from PSUM in <=1024-col chunks
    (double-buffered 2-bank tiles); this is the critical engine (~86us).
  - strips (exp outputs) are RETAINED in SBUF so the PV matmuls for batch
    b can run later, letting QKV of b1 overlap the exp stream of b0.
    PSUM: tag "mm" 4x1 bank (QKV accums / V-transpose / PV accums via slot
    reuse) + tag "sps" 2x2 banks = exactly 8 banks.

Sharding: tensor-parallel over heads.  Core c owns heads {2c, 2c+1}; its
Ws column slice is passed from the host; no collectives.  bf16 inputs with
fp32 accumulation (~4e-3 L2 error vs the fp32 reference).
"""

import numpy as np
import ml_dtypes
from contextlib import ExitStack

import concourse.bass as bass
import concourse.mybir as mybir
import concourse.tile as tile
from concourse import bacc
from concourse.bass_utils import run_bass_kernel_spmd
from concourse.masks import make_identity

B, T, E = 2, 2048, 1024
H, D = 16, 64
NCORES = 8
HPC = H // NCORES            # heads per core = 2
FPC = HPC * D                # feature cols per core per Q/K/V = 128
P = 128
NT = B * T                   # 4096 tokens total
EK = E // P                  # 8 contraction blocks for QKV
KTILES = T // P              # 16 key tiles per batch
DT = mybir.dt.bfloat16
F32 = mybir.dt.float32
SCALE = float(T) ** -0.5
NEG = -1e30
BF = ml_dtypes.bfloat16


def build_kernel():
    nc = bacc.Bacc("TRN2", target_bir_lowering=False, debug=False)
    xt_dram = nc.dram_tensor("xt", (E, NT), DT, kind="ExternalInput")
    w_dram = nc.dram_tensor("wsl", (E, 3 * FPC), DT, kind="ExternalInput")
    # out^T per batch: [FPC, T]; host transposes back.
    out_dram = nc.dram_tensor("outT", (B, FPC, T), F32, kind="ExternalOutput")

    with tile.TileContext(nc) as tc, ExitStack() as ctx:
        const = ctx.enter_context(tc.tile_pool(name="const", bufs=1))
        big = ctx.enter_context(tc.tile_pool(name="big", bufs=1))
        work = ctx.enter_context(tc.tile_pool(name="work", bufs=2))
        strips = ctx.enter_context(tc.tile_pool(name="strips", bufs=1))
        small = ctx.enter_context(tc.tile_pool(name="small", bufs=8))
        outp = ctx.enter_context(tc.tile_pool(name="outp", bufs=4))
        ps = ctx.enter_context(tc.tile_pool(name="ps", bufs=2, space="PSUM"))

        # ---- constants ----
        id_bf = const.tile([P, P], DT, name="id_bf")
        make_identity(nc, id_bf)
        zeros_bf = const.tile([P, 512], DT, name="zeros_bf")
        nc.gpsimd.memset(zeros_bf[:], 0.0)
        # diagmask[p, f] = 0 if f >= p else NEG   (keys on partitions, q free)
        diagmask = const.tile([P, P], F32, name="diagmask")
        nc.gpsimd.memset(diagmask[:], 0.0)
        nc.gpsimd.affine_select(
            out=diagmask[:],
            in_=diagmask[:],
            compare_op=mybir.AluOpType.is_ge,
            fill=NEG,
            base=0,
            pattern=[[1, P]],
            channel_multiplier=-1,
        )

        # ---- weights + xT loads ----
        wsl = big.tile([P, EK, 3 * FPC], DT, name="wsl")
        nc.sync.dma_start(wsl[:], w_dram.rearrange("(eo ei) f -> ei eo f", ei=P))

        xT = big.tile([P, EK, NT], DT, name="xT")
        # slab order must match compute order below: b0 desc, b1 desc
        slab_order = [3, 2, 1, 0, 7, 6, 5, 4]
        for s in slab_order:
            nc.sync.dma_start(
                xT[:, :, 512 * s : 512 * (s + 1)],
                xt_dram[:, 512 * s : 512 * (s + 1)].rearrange(
                    "(eo ei) t -> ei eo t", ei=P
                ),
            )

        qt = big.tile([P, NT], DT, name="qt")
        kt = big.tile([P, NT], DT, name="kt")
        v_nat = big.tile([P, NT // P, P], DT, name="v_nat")  # [tok%128, tb, vfeat]

        # retained per-(b,k,hh) state
        strip_of = {}
        rsumr_of = {}

        def qkv_slab(b, s):
            """Q/K/V^T projection matmuls for 512-token slab s of batch b,
            plus V transposes into v_nat."""
            tok0 = b * T + 512 * s
            for m in range(3):
                acc_ps = ps.tile([P, 512], F32, tag="mm", bufs=4, name="qkv_ps")
                for e in range(EK):
                    nc.tensor.matmul(
                        acc_ps[:],
                        lhsT=wsl[:, e, m * P : (m + 1) * P],
                        rhs=xT[:, e, tok0 : tok0 + 512],
                        start=(e == 0),
                        stop=(e == EK - 1),
                    )
                if m == 0:
                    nc.vector.tensor_copy(qt[:, tok0 : tok0 + 512], acc_ps[:])
                elif m == 1:
                    nc.vector.tensor_copy(kt[:, tok0 : tok0 + 512], acc_ps[:])
                else:
                    vt_s = work.tile([P, 512], DT, tag="vt", bufs=2, name="vt_s")
                    nc.vector.tensor_copy(vt_s[:], acc_ps[:])
                    for w in range(4):
                        tp = ps.tile([P, P], DT, tag="mm", bufs=4, name="vtp")
                        nc.tensor.transpose(
                            tp[:], vt_s[:, w * P : (w + 1) * P], id_bf[:]
                        )
                        nc.vector.tensor_copy(
                            v_nat[:, (b * T // P) + 4 * s + w, :], tp[:]
                        )

        def s_exp(b, k):
            """S^T matmuls + mask + exp for key-tile k of batch b, both heads.
            Strips retained in SBUF; reciprocal row-sums retained."""
            L = T - P * k  # payload cols (q from 128k to T)
            nch = (L + 1023) // 1024
            cls = (L + 511) // 512  # strip size class 1..4
            st = {}
            accs = {0: [], 1: []}
            for hh in range(HPC):
                st[hh] = strips.tile(
                    [P, 512 * cls], DT, tag=f"st{cls}", bufs=(12 if cls <= 2 else 9),
                    name=f"st{cls}",
                )
            for c in range(nch):
                co = 1024 * c
                cw = min(1024, L - co)
                for hh in range(HPC):
                    sps = ps.tile([P, 1024], F32, tag="sps", bufs=2, name="sps")
                    for so in range(0, cw, 512):
                        w = min(512, cw - so)
                        qs = b * T + P * k + co + so
                        nc.tensor.matmul(
                            sps[:, so : so + w],
                            lhsT=kt[hh * D : (hh + 1) * D, b * T + P * k : b * T + P * k + P],
                            rhs=qt[hh * D : (hh + 1) * D, qs : qs + w],
                            start=True,
                            stop=True,
                        )
                    if c == 0:
                        nc.vector.tensor_add(
                            sps[:, 0:P], sps[:, 0:P], diagmask[:]
                        )
                    acc = small.tile([P, 1], F32, tag="acc", name="acc")
                    nc.scalar.activation(
                        st[hh][:, co : co + cw],
                        sps[:, 0:cw],
                        mybir.ActivationFunctionType.Exp,
                        scale=SCALE,
                        accum_out=acc[:],
                    )
                    accs[hh].append(acc)
            for hh in range(HPC):
                if len(accs[hh]) == 1:
                    ssum = accs[hh][0]
                else:
                    ssum = small.tile([P, 1], F32, tag="acc", name="ssum")
                    nc.vector.tensor_add(ssum[:], accs[hh][0][:], accs[hh][1][:])
                rr = small.tile([P, 1], F32, tag="rr", bufs=64, name="rr")
                nc.vector.reciprocal(rr[:], ssum[:])
                strip_of[(b, k, hh)] = st[hh]
                rsumr_of[(b, k, hh)] = rr

        def pv_batch(b):
            """PV accumulation for batch b using retained strips, then evac."""
            pv_ps = [
                ps.tile([P, 512], F32, tag="mm", bufs=4, name=f"pv_{b}_{j}")
                for j in range(4)
            ]
            for j in range(4):
                nc.tensor.matmul(
                    pv_ps[j][:],
                    lhsT=zeros_bf[:, 0:P],
                    rhs=zeros_bf[:],
                    start=True,
                    stop=False,
                    skip_group_check=True,
                )
            for k in range(KTILES - 1, -1, -1):
                q0 = P * k
                for hh in range(HPC):
                    strip = strip_of.pop((b, k, hh))
                    rr = rsumr_of.pop((b, k, hh))
                    vp = small.tile([P, D], DT, tag="vp", bufs=4, name="vp")
                    nc.vector.tensor_scalar_mul(
                        vp[:], v_nat[:, (b * T // P) + k, hh * D : (hh + 1) * D], rr[:]
                    )
                    for j in range(k // 4, 4):
                        lo = max(512 * j, q0)
                        w = 512 * (j + 1) - lo
                        jo = lo - 512 * j
                        nc.tensor.matmul(
                            pv_ps[j][hh * D : (hh + 1) * D, jo : jo + w],
                            lhsT=vp[:],
                            rhs=strip[:, lo - q0 : lo - q0 + w],
                            start=False,
                            stop=(k == 0 and hh == HPC - 1),
                            skip_group_check=True,
                        )
            for j in range(4):
                osb = outp.tile([P, 512], F32, tag="osb", name="osb")
                nc.vector.tensor_copy(osb[:], pv_ps[j][:])
                nc.sync.dma_start(
                    out_dram[b, :, 512 * j : 512 * (j + 1)], osb[:]
                )

        # ---- program order: per-slab QKV interleaved with S/exp so the
        # Scalar engine (exp) is fed from ~10us in and never starves.
        for b in range(B):
            for s in range(3, -1, -1):
                qkv_slab(b, s)
                for k in range(4 * s + 3, 4 * s - 1, -1):
                    s_exp(b, k)
        # PV(b0)'s "mm"-tag slots free once b1's QKV allocations drain, so
        # it runs concurrently with b1's exp stream; PV(b1) trails it.
        pv_batch(0)
        pv_batch(1)

    nc.compile()
    return nc


_NC_CACHE = None


def _build_inputs(x: np.ndarray, Ws: np.ndarray):
    x2 = x.reshape(NT, E)
    xt = np.ascontiguousarray(x2.T).astype(BF)
    in_maps = []
    for c in range(NCORES):
        cols = np.concatenate(
            [
                Ws[:, c * FPC : (c + 1) * FPC],
                Ws[:, E + c * FPC : E + (c + 1) * FPC],
                Ws[:, 2 * E + c * FPC : 2 * E + (c + 1) * FPC],
            ],
            axis=1,
        ).astype(BF)
        in_maps.append({"xt": xt, "wsl": np.ascontiguousarray(cols)})
    return in_maps


def _assemble(results):
    out = np.empty((B, T, H * D), np.float32)
    for c in range(NCORES):
        # results[c]["outT"]: (B, FPC, T) f32
        ot = results[c]["outT"]
        out[:, :, c * FPC : (c + 1) * FPC] = ot.transpose(0, 2, 1)
    return out


def kernel(x: np.ndarray, Ws: np.ndarray) -> np.ndarray:
    global _NC_CACHE
    if _NC_CACHE is None:
        _NC_CACHE = build_kernel()
    nc = _NC_CACHE
    in_maps = _build_inputs(np.asarray(x, np.float32), np.asarray(Ws, np.float32))
    res = run_bass_kernel_spmd(nc, in_maps, core_ids=list(range(NCORES)))
    return _assemble([res.results[c] for c in range(NCORES)])
